# revision 10
# baseline (speedup 1.0000x reference)
"""Trainium2 Bass kernel for nn_LocalAttentionUnFold (local windowed attention).

Math (from the reference):
  q/k/v = x @ W.T + b, reshaped [B,H,S,D] by splitting heads off the
  flattened (S,E) axis => head h's whole sequence lives in rows
  h*128:(h+1)*128 of the [2048,1024] projection matrix, with position
  s at (row h*128 + s//16, col (s%16)*64 + d).
  scores[s,w] = q[s].k[s+w-16] * SCALE (zero-padded k), softmax over w
  (33 raw scores, OOB scores are exactly 0), out[s] = sum_w attn[s,w]*v[s+w-32].

Sharding: core c owns heads 2c,2c+1 == x rows c*256:(c+1)*256; output is
column block [c*128:(c+1)*128] of the final [2048,1024].
"""

import sys

sys.path.insert(0, "/opt/trn_rl_repo")

import numpy as np
import ml_dtypes

import concourse.bass as bass
import concourse.bacc as bacc_mod
import concourse.mybir as mybir
from concourse.tile import TileContext
from concourse.bass_utils import run_bass_kernel_spmd
from concourse.masks import make_identity

B, S, E, H, W = 1, 2048, 1024, 16, 33
D = E // H  # 64
SCALE = float(D) ** -0.5
NCORES = 8
RPC = S // NCORES  # 256 x-rows per core
HPC = H // NCORES  # 2 heads per core
NBLK = S // 128  # 16 seq blocks of 128 per head
GRP = 4  # blocks per attention group

BF16 = mybir.dt.bfloat16
F32 = mybir.dt.float32
AF = mybir.ActivationFunctionType
ALU = mybir.AluOpType

_CACHE = {}


def _build():
    nc = bacc_mod.Bacc()
    xT = nc.declare_dram_parameter("xT", [E, RPC], BF16, isOutput=False)
    wts = {
        t: nc.declare_dram_parameter(f"W{t}T", [E, E], BF16, isOutput=False) for t in ("q", "k", "v")
    }
    bias = {
        t: nc.declare_dram_parameter(f"b{t}", [E], BF16, isOutput=False) for t in ("q", "k", "v")
    }
    maskA_d = nc.declare_dram_parameter("maskA", [32, 128], BF16, isOutput=False)
    maskB_d = nc.declare_dram_parameter("maskB", [128, 128], BF16, isOutput=False)
    out_d = nc.declare_dram_parameter("out", [S, HPC * D], F32, isOutput=True)

    with TileContext(nc) as tc:
        with (
            tc.tile_pool(name="const", bufs=1) as cpool,
            tc.tile_pool(name="qkvt", bufs=1) as tpool,
            tc.tile_pool(name="vtiles", bufs=1) as vpool,
            tc.tile_pool(name="etile", bufs=3) as epool,
            tc.tile_pool(name="osb", bufs=3) as opool,
            tc.tile_pool(name="pproj", bufs=2, space="PSUM") as pproj,
            tc.tile_pool(name="psa", bufs=2, space="PSUM") as psa,
            tc.tile_pool(name="psb", bufs=2, space="PSUM") as psb,
            tc.tile_pool(name="pout", bufs=2, space="PSUM") as pout,
        ):
            # ---- constants into SBUF ----
            xT_sb = cpool.tile([128, 8, RPC], BF16, tag="xT", name="xT_sb")
            nc.sync.dma_start(xT_sb, xT.rearrange("(j p) i -> p j i", p=128))
            w_sb = {}
            b_sb = {}
            for t in ("q", "k", "v"):
                w_sb[t] = cpool.tile([128, 8, E], BF16, tag=f"w{t}", name=f"w_sb_{t}")
                nc.sync.dma_start(w_sb[t], wts[t].rearrange("(j p) o -> p j o", p=128))
                b_sb[t] = cpool.tile([1, E], BF16, tag=f"b{t}", name=f"b_sb_{t}")
                nc.gpsimd.dma_start(b_sb[t], bias[t].rearrange("(a e) -> a e", a=1))
            maskA = cpool.tile([32, 128], BF16, tag="maskA", name="maskA")
            nc.gpsimd.dma_start(maskA, maskA_d[:, :])
            maskB = cpool.tile([128, 128], BF16, tag="maskB", name="maskB")
            nc.gpsimd.dma_start(maskB, maskB_d[:, :])
            ident = cpool.tile([64, 64], BF16, tag="ident", name="ident")
            make_identity(nc, ident)
            ones_i = cpool.tile([1, RPC], BF16, tag="ones_i", name="ones_i")
            nc.gpsimd.memset(ones_i, 1.0)
            # prime the DVE vector clock past the small SWDGE DMAs so later
            # DVE ops carry a single (non-DMA) wait
            prime = cpool.tile([32, 1], BF16, tag="prime", name="prime")
            nc.vector.tensor_copy(out=prime, in_=maskB[:32, 0:1])

            # ---- assembled per-head [d, s'] tensors (both heads stacked) ----
            # QT/KT: s' in [-16, 2064) at free offset 16+s'.  VT: v row r at
            # offset 32+r (rows [-32,0) are the zero pad).
            QT = tpool.tile([64, HPC, 2080], BF16, tag="QT", name="QT")
            KT = tpool.tile([64, HPC, 2080], BF16, tag="KT", name="KT")
            VT = tpool.tile([64, HPC, 2080], BF16, tag="VT", name="VT")
            nc.gpsimd.memset(KT[:, :, 0:16], 0.0)
            nc.gpsimd.memset(KT[:, :, 2064:2080], 0.0)
            nc.gpsimd.memset(VT[:, :, 0:32], 0.0)

            # V in natural [s, d] 128-blocks + ones column for the softmax
            # denominator; V_edge holds the 32 rows [sb-32, sb) per block.
            V_all = [vpool.tile([128, NBLK, D + 1], BF16, tag=f"vall{h}", name=f"vall{h}") for h in range(HPC)]
            V_edge = [vpool.tile([32, NBLK, D + 1], BF16, tag=f"vedge{h}", name=f"vedge{h}") for h in range(HPC)]
            for h in range(HPC):
                nc.gpsimd.memset(V_all[h][:, :, D], 1.0)
                nc.gpsimd.memset(V_edge[h][:, :, D], 1.0)

            dest_of = {"q": (QT, 1), "k": (KT, 1), "v": (VT, 2)}

            # ---- projections: PSUM [o-chunk 128, i 256]; o = s_lo*64 + d so
            # chunk j holds class 2j (parts 0:64) and 2j+1 (parts 64:128),
            # pure d within each half. ----
            for t in ("q", "k", "v"):
                dstT, soff = dest_of[t]
                # view free axis as (s_hi+soff, s_lo): offset soff*16 + 2j+c + 16*s_hi
                dstv = dstT.rearrange("p h (s l) -> p h s l", l=16)
                for j in range(8):
                    ps = pproj.tile([128, RPC], F32, tag="proj", name="ps")
                    for je in range(8):
                        nc.tensor.matmul(
                            ps,
                            lhsT=w_sb[t][:, je, j * 128 : (j + 1) * 128],
                            rhs=xT_sb[:, je, :],
                            start=(je == 0),
                            stop=False,
                        )
                    nc.tensor.matmul(
                        ps,
                        lhsT=b_sb[t][:, j * 128 : (j + 1) * 128],
                        rhs=ones_i,
                        start=False,
                        stop=True,
                    )
                    for c in range(2):
                        src = ps[c * 64 : (c + 1) * 64, :].rearrange(
                            "p (h s) -> p h s", h=HPC
                        )
                        dst = dstv[:, :, soff : soff + 128, 2 * j + c]
                        if t == "q":
                            nc.scalar.activation(dst, src, AF.Copy, scale=SCALE)
                        elif t == "v":
                            nc.scalar.activation(dst, src, AF.Copy)
                        else:
                            nc.vector.tensor_copy(out=dst, in_=src)

            # ---- V transposes: [d, s] -> [s, d] blocks via PE ----
            for h in range(HPC):
                for bb in range(NBLK):
                    sb = bb * 128
                    pt = pproj.tile([128, 64], BF16, tag="proj", name="pt")
                    nc.tensor.transpose(pt, VT[:, h, 32 + sb : 32 + sb + 128], ident)
                    nc.vector.tensor_copy(out=V_all[h][:, bb, :D], in_=pt)
                    pe = pproj.tile([32, 64], BF16, tag="proj", name="pe")
                    nc.tensor.transpose(pe, VT[:, h, sb : sb + 32], ident)
                    nc.scalar.activation(V_edge[h][:, bb, :D], pe, AF.Copy)

            # ---- banded attention, groups of 4 blocks ----
            for h in range(HPC):
                for g in range(NBLK // GRP):
                    sA = psa.tile([32, GRP, 128], F32, tag="sA", name="sA")
                    sB = psb.tile([128, GRP, 128], F32, tag="sB", name="sB")
                    for b in range(GRP):
                        sb = (g * GRP + b) * 128
                        rhs = QT[:, h, sb + 16 : sb + 144]
                        nc.tensor.matmul(
                            sA[:, b, :], lhsT=KT[:, h, sb : sb + 32], rhs=rhs,
                            start=True, stop=True,
                        )
                        nc.tensor.matmul(
                            sB[:, b, :], lhsT=KT[:, h, sb + 32 : sb + 160], rhs=rhs,
                            start=True, stop=True,
                        )
                    eA = epool.tile([32, GRP, 128], BF16, tag="eA", name="eA")
                    eB = epool.tile([128, GRP, 128], BF16, tag="eB", name="eB")
                    nc.scalar.activation(eA, sA, AF.Exp)
                    nc.scalar.activation(eB, sB, AF.Exp)
                    nc.vector.tensor_tensor(
                        eA, eA, maskA[:, None, :].to_broadcast([32, GRP, 128]), ALU.mult
                    )
                    nc.vector.tensor_tensor(
                        eB, eB, maskB[:, None, :].to_broadcast([128, GRP, 128]), ALU.mult
                    )
                    po = pout.tile([128, GRP, D + 1], F32, tag="po", name="po")
                    for b in range(GRP):
                        bb = g * GRP + b
                        nc.tensor.matmul(
                            po[:, b, :], lhsT=eA[:, b, :], rhs=V_edge[h][:, bb, :],
                            start=True, stop=False,
                        )
                        nc.tensor.matmul(
                            po[:, b, :], lhsT=eB[:, b, :], rhs=V_all[h][:, bb, :],
                            start=False, stop=True,
                        )
                    rec = opool.tile([128, GRP], F32, tag="rec", name="rec")
                    nc.vector.reciprocal(rec, po[:, :, D])
                    osb = opool.tile([128, GRP, D], F32, tag="osb", name="osb")
                    nc.vector.tensor_tensor(
                        osb, po[:, :, :D],
                        rec[:, :, None].to_broadcast([128, GRP, D]), ALU.mult,
                    )
                    nc.sync.dma_start(
                        out=out_d[
                            g * GRP * 128 : (g + 1) * GRP * 128,
                            h * D : (h + 1) * D,
                        ].rearrange("(b p) d -> p b d", p=128),
                        in_=osb,
                    )
    nc.finalize()
    return nc


def _masks():
    p = np.arange(32)[:, None]
    f = np.arange(128)[None, :]
    maskA = (f <= p).astype(ml_dtypes.bfloat16)
    p = np.arange(128)[:, None]
    maskB = ((p <= f) & (f <= p + 32)).astype(ml_dtypes.bfloat16)
    return maskA, maskB


def kernel(x, Wq, bq, Wk, bk, Wv, bv):
    if "nc" not in _CACHE:
        _CACHE["nc"] = _build()
    nc = _CACHE["nc"]

    bf = ml_dtypes.bfloat16
    x2 = np.asarray(x, np.float32).reshape(S, E)
    WqT = np.ascontiguousarray(np.asarray(Wq, np.float32).T).astype(bf)
    WkT = np.ascontiguousarray(np.asarray(Wk, np.float32).T).astype(bf)
    WvT = np.ascontiguousarray(np.asarray(Wv, np.float32).T).astype(bf)
    maskA, maskB = _masks()

    in_maps = []
    for c in range(NCORES):
        xTc = np.ascontiguousarray(x2[c * RPC : (c + 1) * RPC, :].T).astype(bf)
        in_maps.append(
            {
                "xT": xTc,
                "WqT": WqT, "WkT": WkT, "WvT": WvT,
                "bq": np.asarray(bq, np.float32).astype(bf),
                "bk": np.asarray(bk, np.float32).astype(bf),
                "bv": np.asarray(bv, np.float32).astype(bf),
                "maskA": maskA, "maskB": maskB,
            }
        )
    res = run_bass_kernel_spmd(nc, in_maps, core_ids=list(range(NCORES))).results
    out = np.concatenate([res[c]["out"] for c in range(NCORES)], axis=1)
    return out.reshape(B, S, E).astype(np.float32)
